# revision 78
# baseline (speedup 1.0000x reference)
"""Trainium2 Bass kernel for nn_ClusterlingLayer (ragged_sequence).

Computes, for B=131072 fibers against K=64 clusters:
  x_dis[b,k] = ||x_b||^2 + ||w_k||^2 - 2 x_b.w_k
  dice[b,k]  = 1 - (2*inter + s)/(nF + nC + s)   (inter = ragged ROI histogram dot)
  q = rownorm( 1 / (1 + x_dis*dice) )
Returns (q, x_dis) like the reference.

Sharding: data-parallel over B across 8 NeuronCores (16384 fibers/core).

Device strategy (v2 - run-length histogram):
 - Host masks invalid rois to sentinel 128 and SORTS each fiber's roi list
   (a pure permutation). On device, run boundaries fall out of two is_equal
   compares of adjacent elements; a single scan-based custom DVE op emits the
   run length at every position, and a second tiny custom op emits the bin
   index at run-ends (-1 elsewhere). One GPSIMD local_scatter per 1024-fiber
   granule then materializes all 8 subtile histograms at once (zero-inits the
   destination, ignores negative indices). This replaces the O(LF*V) compare
   chain of the previous version with O(LF) work per fiber.
 - inter via PE: per-subtile transpose of the histogram (identity matmul,
   PSUM bf16) -> ACT copy -> matmul with the (-2*histC) table.
 - x_dis via PE in fp8(e4m3) with DoubleRow perf mode (two 128-deep
   contraction chunks per instruction) + an fp16 rank-2 augment matmul that
   folds in ||x||^2 (host, exact f32->fp16) and ||w||^2.
 - elementwise: psum_i accumulates den0 - 2*inter exactly in integers (the
   histogram matmul plus a rank-2 [1;len]x[nC;1] matmul), so the reference's
   empty-set special cases fall out of exact-zero cancellation. With
   num = bf16(den0 + s) from the host (== den0 unless den0==0),
   cden = a*xd + num, qn = num * recip(cden), rownorm. The b-multiply runs
   on GPSIMD, the rest on DVE; stages are software-pipelined with 1-3
   granule skew so no engine parks on a cross-engine input.
 - All DRAM layouts are partition-major so every DMA descriptor moves >=512
   contiguous bytes; inputs/outputs ride in 4096-fiber blocks to keep the
   HWDGE/SEQ instruction count small. Outputs travel as bf16 and are upcast
   on the host (tolerance is 2e-2; bf16 adds ~4e-3 worst-case).
"""

import os
import sys

import numpy as np

for _p in ("/opt/trn_rl_repo", os.path.expanduser("~/.axon_site/_ro/trn_rl_repo")):
    if os.path.isdir(_p) and _p not in sys.path:
        sys.path.insert(0, _p)

import concourse.bass as bass
import concourse.mybir as mybir
import concourse.tile as tile
from concourse import bacc
from concourse.bass_utils import run_bass_kernel_spmd

import ml_dtypes

NCORES = 8
B, D, K, LF, LC = 131072, 512, 64, 24, 64
V = 128              # ROI vocab; also the sentinel value for masked slots
BS = B // NCORES     # fibers per core
T = BS // 128        # subtiles (of 128 fibers) per core
NBLK = 4             # DMA blocks per core
TB = T // NBLK       # subtiles per block (32)
NGR = 4              # granules per block
SG = TB // NGR       # subtiles per granule (8)
WROW = LF + 2        # rl row: [pad -7 | 24 sorted rois | pad 200]
HPAGE = 130          # hist page: 128 bins + sentinel + pad
HTOT = SG * HPAGE    # scatter width per granule (1040)
PREP = TB * LF       # prep op width per block (768)
SMOOTH = 1e-6

f32 = mybir.dt.float32
bf16 = mybir.dt.bfloat16
f16 = mybir.dt.float16
fp8 = mybir.dt.float8e4
i16 = mybir.dt.int16


def _register_custom_ops():
    """Register the two run-length DVE ops (scan-based count, index select).
    Self-pins the uop shas like the stock custom ops do."""
    from concourse import dve_ops
    from concourse.dve_spec import (
        Spec, Src0, Src1, One, MaxNeg, select, scan, lower, AluOp,
        _has_src1 as has_src1,
    )

    if "RLCNT_ANT" in dve_ops._SUB_OPCODE_FOR_NAME:
        return

    def _cnt_ref(in0, in1, s0, s1, imm2):
        m = np.maximum.accumulate(
            np.where(in0 != 0, np.float32(-3.4e38), in1.astype(np.float32)), axis=-1
        )
        return (in1.astype(np.float32) - m + 1.0).astype(np.float32)

    cnt = dve_ops.DveOp(
        "RLCNT_ANT",
        Spec(
            # in0 = eq(r_j, r_{j-1}) ("not a run start"), in1 = iota.
            # out_j = j - (index of current run start) + 1  == run length so far
            body=Src1 - scan(AluOp.MAX, select(Src0, MaxNeg, Src1)) + One,
            reference=_cnt_ref,
        ),
        subdim=False,
        uops_sha={},
    )
    idx = dve_ops.DveOp(
        "RLIDX_ANT",
        Spec(
            # in0 = roi + 130*page + 1, in1 = eq(r_j, r_{j+1}) ("not a run end")
            # out_j = roi + 130*page at run ends, -1 elsewhere
            body=Src0 * (One - Src1) - One,
            reference=lambda in0, in1, s0, s1, imm2: (
                in0.astype(np.float32) * (1.0 - in1.astype(np.float32)) - 1.0
            ).astype(np.float32),
        ),
        subdim=False,
        uops_sha={},
    )
    for op in (cnt, idx):
        dve_ops.OPS.append(op)
        dve_ops.CUSTOM_DVE_SPECS[op.name] = op.spec
        dve_ops._SUB_OPCODE_FOR_NAME[op.name] = (
            max(dve_ops._SUB_OPCODE_FOR_NAME.values()) + 1
        )
    for op in (cnt, idx):
        for ver in ("v3", "v4"):
            spec_c = dve_ops.DveOpSpec(
                name=op.name,
                opcode=dve_ops.get_dve_sub_opcode(op.name),
                uops=lower(op.spec, ver=ver),
                rd1_en=has_src1(op.spec),
            )
            op.uops_sha[ver] = spec_c.sha(ver)


def _build_nc():
    _register_custom_ops()
    from concourse.dve_ops import OPS as _OPS
    CNT_OP = next(o for o in _OPS if o.name == "RLCNT_ANT")
    IDX_OP = next(o for o in _OPS if o.name == "RLIDX_ANT")

    nc = bacc.Bacc("TRN2", target_bir_lowering=False)

    xr = nc.dram_tensor("xr", [128, 4 * BS], fp8, kind="ExternalInput")
    rlr = nc.dram_tensor("rlr", [128, T * WROW], i16, kind="ExternalInput")
    d0r = nc.dram_tensor("d0r", [128, T * K], bf16, kind="ExternalInput")
    wT = nc.dram_tensor("wT", [128, 4 * K], fp8, kind="ExternalInput")
    # merged constant uploads (fewer serial HWDGE round-trips at startup):
    augs = nc.dram_tensor("augs", [2, 2 * BS + 2 * K], f16,
                          kind="ExternalInput")   # [wsq;1 | 1;xsq | 1;len | nC;1]
    it = nc.dram_tensor("it", [128, 128 + K], bf16,
                        kind="ExternalInput")     # [ident | tbl]
    pl = nc.dram_tensor("pl", [2 * PREP], f16,
                        kind="ExternalInput")     # [pgc | lin]

    qr = nc.dram_tensor("qr", [128, T * K], bf16, kind="ExternalOutput")
    xdr = nc.dram_tensor("xdr", [128, T * K], bf16, kind="ExternalOutput")

    xr_v = xr[:].rearrange("p (c b) -> p c b", c=4)
    rl_v = rlr[:].rearrange("p (t j) -> p t j", j=WROW)
    d0_v = d0r[:].rearrange("p (t k) -> p t k", k=K)
    q_v = qr[:].rearrange("p (t k) -> p t k", k=K)
    xd_v = xdr[:].rearrange("p (t k) -> p t k", k=K)

    def bcast_row(dram_ap, n):
        return bass.AP(
            tensor=dram_ap.tensor,
            offset=dram_ap.offset,
            ap=[[0, n]] + dram_ap.ap,
        )

    DR = mybir.MatmulPerfMode.DoubleRow

    with tile.TileContext(nc) as tc:
        with (
            tc.tile_pool(name="consts", bufs=1) as consts,
            tc.tile_pool(name="xin", bufs=5) as xin,
            tc.tile_pool(name="rin", bufs=1) as rin,
            tc.tile_pool(name="din", bufs=1) as din,
            tc.tile_pool(name="prep", bufs=4) as prep,
            tc.tile_pool(name="hist", bufs=6) as hist,
            tc.tile_pool(name="htp", bufs=5) as htp,
            tc.tile_pool(name="ew", bufs=6) as ew,
            tc.tile_pool(name="outs", bufs=2) as outs,
            tc.tile_pool(name="psx", bufs=3, space="PSUM") as psx,
            tc.tile_pool(name="psi", bufs=3, space="PSUM") as psi,
            tc.tile_pool(name="pst", bufs=2, space="PSUM") as pst,
        ):
            c_wT = consts.tile([128, 4, K], fp8)
            c_augs = consts.tile([2, 2 * BS + 2 * K], f16)
            c_wsq1 = c_augs[:, 0:K]
            c_aug = c_augs[:, K:K + BS]
            c_laug = c_augs[:, K + BS:K + 2 * BS]
            c_lnc1 = c_augs[:, K + 2 * BS:2 * K + 2 * BS]
            c_it = consts.tile([128, 128 + K], bf16)
            c_id = c_it[:, 0:128]
            c_tbl = c_it[:, 128:128 + K]
            c_pl = consts.tile([128, 2 * PREP], f16)
            c_pgc = c_pl[:, 0:PREP]
            c_lin = c_pl[:, PREP:2 * PREP]

            NG = NBLK * NGR           # total granules
            binfo = {}                # blk -> dict of block tiles
            ginfo = {}                # g -> dict (H, pt, hT)

            # all input DMAs issued up front: they carry no waits, so the SP
            # queue never parks an out-DMA wait in front of a needed input.
            # rl/den0 first (small, unblock prep+scatter), then x by granule
            # so the first matmuls don't wait on a whole-block x transfer.
            NG_ = NBLK * NGR
            indma = {}
            xg = {}

            def issue_xdma(g):
                # one granule of x (4 KB/partition), prefetched ~2 ahead
                xt = xin.tile([128, 4, SG * 128], fp8, tag="xg")
                f0 = g * SG * 128
                nc.sync.dma_start(out=xt, in_=xr_v[:, :, f0:f0 + SG * 128])
                xg[g] = xt

            # hot-path-first DMA order: block-0 prep deps, consts, first x
            rt = rin.tile([128, TB, WROW], i16, tag="rt0")
            nc.sync.dma_start(out=rt, in_=rl_v[:, 0:TB, :])
            nc.sync.dma_start(out=c_pl, in_=bcast_row(pl[:], 128))
            nm = din.tile([128, TB, K], bf16, tag="nm0")
            nc.sync.dma_start(out=nm, in_=d0_v[:, 0:TB, :])
            indma[0] = [rt, nm]
            nc.sync.dma_start(out=c_it, in_=it[:])
            nc.sync.dma_start(
                out=c_wT, in_=wT[:].rearrange("p (c k) -> p c k", c=4))
            nc.sync.dma_start(out=c_augs, in_=augs[:])
            issue_xdma(0)
            rt = rin.tile([128, TB, WROW], i16, tag="rt1")
            nc.sync.dma_start(out=rt, in_=rl_v[:, TB:2 * TB, :])
            nm = din.tile([128, TB, K], bf16, tag="nm1")
            nc.sync.dma_start(out=nm, in_=d0_v[:, TB:2 * TB, :])
            indma[1] = [rt, nm]
            issue_xdma(1)
            for blk in range(2, NBLK):
                t0 = blk * TB
                rt = rin.tile([128, TB, WROW], i16, tag=f"rt{blk}")
                nc.sync.dma_start(out=rt, in_=rl_v[:, t0:t0 + TB, :])
                nm = din.tile([128, TB, K], bf16, tag=f"nm{blk}")
                nc.sync.dma_start(out=nm, in_=d0_v[:, t0:t0 + TB, :])
                indma[blk] = [rt, nm]

            def issue_block_a(blk, part=None):
                # run-length prep; part=None does the whole block, part=0/1
                # split it in halves so the first scatters can start earlier
                rt, nm = indma[blk]
                if part in (None, 0):
                    z = prep.tile([128, TB, LF + 1], bf16, tag="z")
                    rr = prep.tile([128, TB, LF], f16, tag="rr")
                    cnt = prep.tile([128, PREP], bf16, tag="cnt")
                    idxt = prep.tile([128, PREP], i16, tag="idx")
                    qb = outs.tile([128, TB, K], bf16, tag="qb")
                    xdb = outs.tile([128, TB, K], bf16, tag="xdb")
                    binfo[blk] = dict(
                        num=nm, cnt=cnt, idxt=idxt, qb=qb, xdb=xdb,
                        z=z, rr=rr)
                    ts_, te_ = 0, (TB if part is None else TB // 2)
                else:
                    ts_, te_ = TB // 2, TB
                bi = binfo[blk]
                z, rr, cnt, idxt = bi["z"], bi["rr"], bi["cnt"], bi["idxt"]
                j0, j1 = ts_ * LF, te_ * LF
                nc.vector.tensor_tensor(
                    out=z[:, ts_:te_, :], in0=rt[:, ts_:te_, 1:LF + 2],
                    in1=rt[:, ts_:te_, 0:LF + 1],
                    op=mybir.AluOpType.is_equal)
                nc.vector.tensor_tensor(
                    out=rr[:, ts_:te_, :], in0=rt[:, ts_:te_, 1:LF + 1],
                    in1=c_pgc[:].rearrange(
                        "p (t j) -> p t j", j=LF)[:, ts_:te_, :],
                    op=mybir.AluOpType.add)
                nc.vector._custom_dve(
                    CNT_OP,
                    out=cnt[:].rearrange(
                        "p (t j) -> p t j", j=LF)[:, ts_:te_, :],
                    in0=z[:, ts_:te_, 0:LF], in1=c_lin[:, j0:j1])
                nc.vector._custom_dve(
                    IDX_OP,
                    out=idxt[:].rearrange(
                        "p (t j) -> p t j", j=LF)[:, ts_:te_, :],
                    in0=rr[:, ts_:te_, :], in1=z[:, ts_:te_, 1:LF + 1])

            def issue_scatter(g):
                # all 8 subtile histograms of granule g in one scatter
                bi = binfo[g // NGR]
                j0 = (g % NGR) * SG * LF
                H = hist.tile([128, HTOT], bf16, tag="H")
                nc.gpsimd.local_scatter(
                    out_ap=H[:],
                    data_ap=bi["cnt"][:, j0:j0 + SG * LF],
                    idxs_ap=bi["idxt"][:, j0:j0 + SG * LF],
                    channels=128, num_elems=HTOT, num_idxs=SG * LF,
                )
                ginfo[g] = dict(H=H)

            def issue_transposes(g):
                H = ginfo[g]["H"]
                pt = pst.tile([128, SG, 128], bf16, tag="pt")
                for s in range(SG):
                    nc.tensor.transpose(
                        out=pt[:, s, :],
                        in_=H[:, HPAGE * s:HPAGE * s + V], identity=c_id)
                hT = htp.tile([128, SG, 128], bf16, tag="hT")
                nc.scalar.copy(out=hT, in_=pt)
                ginfo[g]["hT"] = hT

            def issue_b(g):
                blk, gl = divmod(g, NGR)
                bi = binfo[blk]
                s0 = gl * SG
                bb = ew.tile([128, SG, K], bf16, tag="b")
                nc.gpsimd.tensor_tensor(
                    out=bb, in0=ginfo[g]["aT"],
                    in1=bi["xdb"][:, s0:s0 + SG, :], op=mybir.AluOpType.mult)
                ginfo[g]["b"] = bb

            def issue_cden_rc(g):
                blk, gl = divmod(g, NGR)
                bi = binfo[blk]
                s0 = gl * SG
                cden = ew.tile([128, SG, K], bf16, tag="c")
                nc.vector.tensor_tensor(
                    out=cden, in0=ginfo[g]["b"],
                    in1=bi["num"][:, s0:s0 + SG, :], op=mybir.AluOpType.add)
                rc = ew.tile([128, SG, K], bf16, tag="rc")
                with nc.allow_low_precision("q tolerance is 2e-2"):
                    nc.vector.reciprocal(out=rc, in_=cden)
                qn = ew.tile([128, SG, K], bf16, tag="qn")
                nc.vector.tensor_tensor(
                    out=qn, in0=bi["num"][:, s0:s0 + SG, :], in1=rc,
                    op=mybir.AluOpType.mult)
                ginfo[g]["qn"] = qn
                # half-width add before the 1x-mode reduce
                qh = ew.tile([128, SG, K // 2], bf16, tag="qh")
                nc.vector.tensor_tensor(
                    out=qh, in0=qn[:, :, 0:K // 2], in1=qn[:, :, K // 2:K],
                    op=mybir.AluOpType.add)
                rs = ew.tile([128, SG], f32, tag="rs")
                with nc.allow_low_precision("q tolerance is 2e-2"):
                    nc.vector.tensor_reduce(
                        out=rs, in_=qh, axis=mybir.AxisListType.X,
                        op=mybir.AluOpType.add)
                ginfo[g]["rs"] = rs

            def issue_norm(g):
                blk, gl = divmod(g, NGR)
                bi = binfo[blk]
                s0 = gl * SG
                t0 = blk * TB
                qn = ginfo[g]["qn"]
                rs = ginfo[g]["rs"]
                rn = ew.tile([128, SG], f32, tag="rn")
                nc.vector.reciprocal(out=rn, in_=rs)
                # duplicate rn into pairs so the broadcast AP below has a
                # stride-1 last dim: that keeps the qf multiply in 2x mode
                rn2 = ew.tile([128, SG, 2], bf16, tag="rn2")
                rn_ap = rn[:]
                nc.vector.tensor_copy(
                    out=rn2,
                    in_=bass.AP(
                        tensor=rn_ap.tensor, offset=rn_ap.offset,
                        ap=list(rn_ap.ap) + [[0, 2]],
                    ))
                rn2_ap = rn2[:]
                rn2_b = bass.AP(
                    tensor=rn2_ap.tensor, offset=rn2_ap.offset,
                    ap=[rn2_ap.ap[0], [2, SG], [0, K // 2], [1, 2]],
                )
                nc.vector.tensor_tensor(
                    out=bi["qb"][:, s0:s0 + SG, :], in0=qn, in1=rn2_b,
                    op=mybir.AluOpType.mult)
                if blk == NBLK - 1:
                    # last block: drain outputs per granule to shorten the
                    # pipeline tail
                    f0 = t0 + s0
                    nc.sync.dma_start(
                        out=q_v[:, f0:f0 + SG, :],
                        in_=bi["qb"][:, s0:s0 + SG, :])
                    nc.sync.dma_start(
                        out=xd_v[:, f0:f0 + SG, :],
                        in_=bi["xdb"][:, s0:s0 + SG, :])
                elif gl == NGR - 1:
                    h = TB // 2
                    nc.sync.dma_start(
                        out=q_v[:, t0:t0 + h, :], in_=bi["qb"][:, 0:h, :])
                    nc.sync.dma_start(
                        out=q_v[:, t0 + h:t0 + TB, :], in_=bi["qb"][:, h:TB, :])
                    nc.sync.dma_start(
                        out=xd_v[:, t0:t0 + h, :], in_=bi["xdb"][:, 0:h, :])
                    nc.sync.dma_start(
                        out=xd_v[:, t0 + h:t0 + TB, :],
                        in_=bi["xdb"][:, h:TB, :])

            # software-pipelined prologue
            issue_block_a(0, part=0)
            issue_scatter(0)
            issue_scatter(1)
            issue_block_a(0, part=1)
            issue_transposes(0)
            issue_block_a(1)

            for g in range(NG):
                blk, gl = divmod(g, NGR)
                bi = binfo[blk]
                t0 = blk * TB
                s0 = gl * SG

                if g + 2 < NG:
                    issue_xdma(g + 2)
                if g > 0:
                    issue_b(g - 1)          # Pool: ahead of the scatter

                hT = ginfo[g]["hT"]
                psum_i = psi.tile([128, SG, K], f32, tag="pi")
                psum_x = psx.tile([128, SG, K], f32, tag="px")
                # all x-matmuls before the hist matmuls: they only need the
                # x DMA, so the in-order PE queue never parks them behind a
                # histogram transpose/copy chain
                for s in range(SG):
                    gofs = (t0 + s0 + s) * 128  # fiber offset in core
                    for c2 in (0, 2):
                        nc.tensor.matmul(
                            psum_x[:, s, :],
                            lhsT=xg[g][:, c2:c2 + 2, s * 128:(s + 1) * 128],
                            rhs=c_wT[:, c2:c2 + 2, :],
                            start=(c2 == 0), stop=False, perf_mode=DR)
                    nc.tensor.matmul(
                        psum_x[:, s, :],
                        lhsT=c_aug[:, gofs:gofs + 128], rhs=c_wsq1,
                        start=False, stop=True)
                for s in range(SG):
                    gofs = (t0 + s0 + s) * 128
                    nc.tensor.matmul(
                        psum_i[:, s, :], lhsT=hT[:, s, :], rhs=c_tbl,
                        start=True, stop=False)
                    nc.tensor.matmul(
                        psum_i[:, s, :],
                        lhsT=c_laug[:, gofs:gofs + 128], rhs=c_lnc1,
                        start=False, stop=True)

                # ---- elementwise (deep-skewed pipeline) ----
                # xd copy first: psum_x completes well before psum_i
                nc.scalar.copy(out=bi["xdb"][:, s0:s0 + SG, :], in_=psum_x)
                aT = ew.tile([128, SG, K], f16, tag="aT")
                nc.scalar.copy(out=aT, in_=psum_i)  # den0 - 2*inter, exact
                ginfo[g]["aT"] = aT

                # each stage consumes inputs produced >= 1 iteration ago, so
                # no engine parks at its queue head on a cross-engine input
                if g > 1:
                    issue_cden_rc(g - 2)    # DVE (cden, rc, qn, rs)
                if g > 2:
                    issue_norm(g - 3)       # DVE (+ out-DMA at block ends)
                # block lookahead: prep for blk+2 early in blk (block 1's
                # prep is already in the prologue)
                if gl == 1 and blk + 2 < NBLK:
                    issue_block_a(blk + 2)
                if g == NG - 1:
                    issue_b(g)
                    issue_cden_rc(g - 1)
                    issue_cden_rc(g)
                    for gg in (g - 2, g - 1, g):
                        issue_norm(gg)
                # slack-bearing prefetches at the END of each engine's
                # per-iteration program: scatter(g+2) on Pool (behind b),
                # transposes/hT(g+1) on PE/ACT (behind mm and the copies)
                # prefetch ramps from depth 2 to depth 3 over the first
                # two iterations: a full-depth backlog at t=0 would park
                # granule-0's b/cden behind 6.5us of serial Pool scatters
                if g == 0:
                    issue_scatter(2)
                    issue_transposes(1)
                elif g == 1:
                    issue_scatter(3)
                    issue_scatter(4)
                    issue_transposes(2)
                    issue_transposes(3)
                elif g + 3 < NG:
                    issue_scatter(g + 3)
                if g >= 2 and g + 2 < NG:
                    issue_transposes(g + 2)

            del ginfo

    nc.finalize()
    return nc


_NC_CACHE = None
_LAST = None


def _get_nc():
    global _NC_CACHE
    if _NC_CACHE is None:
        _NC_CACHE = _build_nc()
    return _NC_CACHE


def _pmajor(arr, width):
    """[BS, width] -> [128, T*width] with fiber = t*128 + p."""
    return np.ascontiguousarray(
        arr.reshape(T, 128, width).transpose(1, 0, 2).reshape(128, T * width))


def _unpmajor(arr, width):
    """[128, T*width] -> [BS, width]."""
    return arr.reshape(128, T, width).transpose(1, 0, 2).reshape(BS, width)


def kernel(x, weight, fiber_rois, fiber_lens, cluster_rois, cluster_lens):
    x = np.asarray(x, np.float32)
    weight = np.asarray(weight, np.float32)
    fiber_rois = np.asarray(fiber_rois, np.int32)
    fiber_lens = np.asarray(fiber_lens, np.int32)
    cluster_rois = np.asarray(cluster_rois, np.int32)
    cluster_lens = np.asarray(cluster_lens, np.int32)

    e4m3 = ml_dtypes.float8_e4m3
    bfl = ml_dtypes.bfloat16

    # ---- K-side host prep (tiny) ----
    mC = (np.arange(LC)[None, :] < cluster_lens[:, None])
    histC = np.zeros((K, V), np.float32)
    for k in range(K):
        histC[k] = np.bincount(cluster_rois[k][mC[k]], minlength=V)
    wsq = (weight * weight).sum(1).astype(np.float32)
    wT2 = (-2.0 * weight.T).astype(e4m3)                    # [512, 64]
    wT_r = np.ascontiguousarray(
        wT2.reshape(4, 128, K).transpose(1, 0, 2).reshape(128, 4 * K))
    nC = cluster_lens.astype(np.float32)

    pgc = np.repeat(
        (HPAGE * (np.arange(TB) % SG) + 1).astype(np.float32), LF)
    lin = np.arange(PREP).astype(np.float32)
    pl = np.concatenate([pgc, lin]).astype(np.float16)
    it = np.concatenate(
        [np.eye(128, dtype=np.float32), -2.0 * histC.T], axis=1).astype(bfl)

    jj = np.arange(LF)[None, :]
    nc_bass = _get_nc()
    in_maps = []
    for ci in range(NCORES):
        sl = slice(ci * BS, (ci + 1) * BS)
        xs = x[sl]
        xsq = np.einsum("bd,bd->b", xs, xs).astype(np.float32)
        xq = xs.T.astype(e4m3)                              # [512, BS]
        xr = np.ascontiguousarray(
            xq.reshape(4, 128, BS).transpose(1, 0, 2).reshape(128, 4 * BS))
        lens = fiber_lens[sl]
        one = np.ones(BS, np.float32)
        augs = np.concatenate([
            np.stack([wsq, np.ones(K, np.float32)], 0),
            np.stack([one, xsq], 0),
            np.stack([one, lens.astype(np.float32)], 0),
            np.stack([cluster_lens.astype(np.float32),
                      np.ones(K, np.float32)], 0),
        ], axis=1).astype(np.float16)
        mr = np.where(jj < lens[:, None], fiber_rois[sl], V).astype(np.int16)
        mr.sort(axis=1)                                     # sorted, 128s last
        rl = np.empty((BS, WROW), np.int16)
        rl[:, 0] = -7
        rl[:, 1:LF + 1] = mr
        rl[:, LF + 1] = 200

        numh = (lens.astype(np.float32)[:, None] + nC[None, :]
                + SMOOTH).astype(bfl)

        in_maps.append({
            "xr": xr,
            "rlr": _pmajor(rl, WROW),
            "d0r": _pmajor(numh, K),
            "wT": wT_r,
            "augs": augs,
            "it": it,
            "pl": pl,
        })

    res = run_bass_kernel_spmd(nc_bass, in_maps, core_ids=list(range(NCORES)))
    global _LAST
    _LAST = res
    q = np.concatenate(
        [_unpmajor(r["qr"], K) for r in res.results], axis=0).astype(np.float32)
    xd = np.concatenate(
        [_unpmajor(r["xdr"], K) for r in res.results], axis=0).astype(np.float32)
    return (q, xd)
